# revision 9
# baseline (speedup 1.0000x reference)
"""Trainium2 Bass kernel for nn_MemoryRetriever (retrieval_knn).

Strategy: pure data parallel over batch B across 8 NeuronCores (256 batches
each).  Host pre-transposes memory to [B, D, M] so each batch-pair is one
contiguous 1 MiB DMA into SBUF as a [128, 2048] tile (two batches stacked on
the partition dim: even batch on partitions 0-63, odd on 64-127).

Per core, batches are processed in groups of 32 (16 pairs):
  - einsum1 (sims = q_proj . mem / 8): PE matmuls with a growing-prefix
    block-diagonal stationary (zero-padded q columns) scatter-accumulate the
    sims of all 32 batches of a group into one PSUM tile S[32, 2048] --
    directly in the batched-softmax layout.
  - softmax: DVE reduce_max (negated) -> ACT exp with per-partition bias and
    fused accum_out (sum of exp).  Weights kept unnormalized; 1/sumexp is
    folded into the final per-batch output scale.
  - einsum2 (retrieved = sum_m w*mem): PE broadcasts each pair's two weight
    rows to all 128 partitions via a precomputed selector matmul, then DVE
    scalar_tensor_tensor with fused accum_out does the weighted reduction
    against the resident memT tile (one on-chip pass, no second HBM read).
  - gate MLP + output matmuls on PE/ACT; sigmoid via exp (one ACT table set).
  - device returns gate and (gate/sumexp) * (retU @ Wo @ Wc); the rank-1
    bias/null terms (gate*bo@Wc + (1-gate)*null@Wc + bc) are added on host.
"""

import os
import sys
from contextlib import ExitStack

for _p in ("/opt/trn_rl_repo", "/opt/pypackages"):
    if _p not in sys.path and os.path.isdir(_p):
        sys.path.append(_p)

import numpy as np

import concourse.bass as bass
import concourse.mybir as mybir
import concourse.tile as tile
from concourse import bacc, masks
from concourse.bass_utils import run_bass_kernel_spmd

B, M, H, D, V = 2048, 2048, 64, 64, 128
NCORES = 8
BC = B // NCORES            # 256 batches per core
GRP = 32                    # batches per group
NGRP = BC // GRP            # 8 groups
PAIRS = GRP // 2            # 16 pairs per group
NPAIR_TOT = BC // 2         # 128 pair tiles per core
MEM_BUFS = 18               # SBUF slots for 1 MiB memT pair tiles

F32 = mybir.dt.float32
AF = mybir.ActivationFunctionType
ALU = mybir.AluOpType
AX = mybir.AxisListType

_CACHE = {}
LAST_RESULTS = None         # BassKernelResults of the last run (for profiling)


def build_module():
    nc = bacc.Bacc("TRN2", target_bir_lowering=False, debug=False)

    memT = nc.dram_tensor("memT", [NPAIR_TOT, 128, M], F32, kind="ExternalInput").ap()
    qT = nc.dram_tensor("qT", [H, BC], F32, kind="ExternalInput").ap()
    Wq_d = nc.dram_tensor("Wq", [H, D], F32, kind="ExternalInput").ap()
    bq_d = nc.dram_tensor("bq_s", [D, 1], F32, kind="ExternalInput").ap()
    Wo_d = nc.dram_tensor("Wo", [D, H], F32, kind="ExternalInput").ap()
    Wg1_d = nc.dram_tensor("Wg1", [H + 1, 32], F32, kind="ExternalInput").ap()
    bg1_d = nc.dram_tensor("bg1", [32, 1], F32, kind="ExternalInput").ap()
    Wg2_d = nc.dram_tensor("Wg2", [32, 1], F32, kind="ExternalInput").ap()
    nbg2_d = nc.dram_tensor("nbg2", [1, 1], F32, kind="ExternalInput").ap()
    Wc_d = nc.dram_tensor("Wc", [H, V], F32, kind="ExternalInput").ap()
    logits_o = nc.dram_tensor("logits", [BC, V], F32, kind="ExternalOutput").ap()
    gate_o = nc.dram_tensor("gate", [1, BC], F32, kind="ExternalOutput").ap()

    with tile.TileContext(nc) as tc, ExitStack() as ctx:
        consts = ctx.enter_context(tc.tile_pool(name="consts", bufs=1))
        tp_pool = ctx.enter_context(tc.tile_pool(name="tp", bufs=1))
        mem_pool = ctx.enter_context(tc.tile_pool(name="mem", bufs=MEM_BUFS))
        w_pool = ctx.enter_context(tc.tile_pool(name="W", bufs=2))
        scr_pool = ctx.enter_context(tc.tile_pool(name="scr", bufs=2))
        acc_pool = ctx.enter_context(tc.tile_pool(name="acc", bufs=4))
        retc_pool = ctx.enter_context(tc.tile_pool(name="retc", bufs=2))
        small = ctx.enter_context(tc.tile_pool(name="small", bufs=2))
        s_pool = ctx.enter_context(tc.tile_pool(name="S", bufs=1, space="PSUM"))
        wb_pool = ctx.enter_context(tc.tile_pool(name="wb", bufs=2, space="PSUM"))

        # ---- constants / weights in SBUF ----
        ident = consts.tile([128, 128], F32, tag="ident")
        masks.make_identity(nc, ident[:])

        ones1 = consts.tile([1, 1], F32, tag="ones1")
        nc.gpsimd.memset(ones1[:], 1.0)

        # SEL[k, p*128 + j] = 1 iff k == 2p + (j >= 64).  lhsT slice for the
        # per-pair weight-broadcast matmul: Wb = SEL_p.T @ W.
        sel = consts.tile([GRP, PAIRS * 128], F32, tag="sel")
        nc.gpsimd.memset(sel[:], 0.0)
        sel4 = sel[:].rearrange("k (p h j) -> k p h j", p=PAIRS, h=2, j=64)
        nc.gpsimd.affine_select(
            out=sel4, in_=sel4,
            pattern=[[-2, PAIRS], [-1, 2], [0, 64]],
            compare_op=ALU.not_equal, fill=1.0,
            base=0, channel_multiplier=1,
        )

        wq_sb = consts.tile([H, D], F32, tag="wq")
        nc.sync.dma_start(wq_sb[:], Wq_d)
        # bq (pre-scaled by 1/8 on host), replicated on both partition halves
        bq2 = consts.tile([128, 1], F32, tag="bq2")
        nc.sync.dma_start(bq2[0:64, :], bq_d)
        nc.sync.dma_start(bq2[64:128, :], bq_d)
        wo_sb = consts.tile([D, H], F32, tag="wo")
        nc.sync.dma_start(wo_sb[:], Wo_d)
        wg1_sb = consts.tile([H, 32], F32, tag="wg1")
        nc.sync.dma_start(wg1_sb[:], Wg1_d[0:64, :])
        wg1r_sb = consts.tile([1, 32], F32, tag="wg1r")
        nc.sync.dma_start(wg1r_sb[:], Wg1_d[64:65, :])
        bg1_sb = consts.tile([32, 1], F32, tag="bg1")
        nc.sync.dma_start(bg1_sb[:], bg1_d)
        wg2_sb = consts.tile([32, 1], F32, tag="wg2")
        nc.sync.dma_start(wg2_sb[:], Wg2_d)
        nbg2_sb = consts.tile([1, 1], F32, tag="nbg2")
        nc.sync.dma_start(nbg2_sb[:], nbg2_d)
        wc_sb = consts.tile([H, V], F32, tag="wc")
        nc.sync.dma_start(wc_sb[:], Wc_d)
        qt_sb = consts.tile([H, BC], F32, tag="qt")
        nc.sync.dma_start(qt_sb[:], qT)

        # Growing-prefix block-diagonal stationaries.  T[0] is full width
        # (32 cols) so its start=True matmul clears the whole S tile; T[p]
        # has 2p leading zero columns and the pair's two q columns.
        Tp = []
        for p in range(PAIRS):
            w = GRP if p == 0 else 2 * p + 2
            t = tp_pool.tile([128, w], F32, tag=f"T{p}")
            nc.gpsimd.memset(t[:], 0.0)
            Tp.append(t)
        qblk = tp_pool.tile([128, GRP], F32, tag="qblk")
        nc.gpsimd.memset(qblk[:], 0.0)

        for g in range(NGRP):
            c0 = g * GRP

            # ---- q_proj (scaled by 1/8), duplicated on both halves ----
            qp_ps = wb_pool.tile([128, GRP], F32, tag="wb")
            nc.tensor.matmul(qp_ps[0:64, :], wq_sb[:], qt_sb[:, c0 : c0 + GRP],
                             start=True, stop=True)
            nc.tensor.matmul(qp_ps[64:128, :], wq_sb[:], qt_sb[:, c0 : c0 + GRP],
                             start=True, stop=True, tile_position=(0, 64))
            qp2 = small.tile([128, GRP], F32, tag="qp2")
            nc.scalar.activation(qp2[0:64, :], qp_ps[0:64, :], AF.Identity,
                                 bias=bq2[0:64, :], scale=0.125)
            nc.scalar.activation(qp2[64:128, :], qp_ps[64:128, :], AF.Identity,
                                 bias=bq2[64:128, :], scale=0.125)

            # block-diagonal q columns: even batches on top half, odd on bottom
            nc.vector.tensor_copy(qblk[0:64, 0::2], qp2[0:64, 0::2])
            nc.vector.tensor_copy(qblk[64:128, 1::2], qp2[64:128, 1::2])
            for p in range(PAIRS):
                nc.vector.tensor_copy(Tp[p][:, 2 * p : 2 * p + 2],
                                      qblk[:, 2 * p : 2 * p + 2])

            # ---- einsum1: S[b, m] for the whole group ----
            S = s_pool.tile([GRP, M], F32, tag="S")
            mts = []
            for p in range(PAIRS):
                mt = mem_pool.tile([128, M], F32, tag="memT")
                nc.sync.dma_start(mt[:], memT[g * PAIRS + p])
                mts.append(mt)
                rows = GRP if p == 0 else 2 * p + 2
                for c in range(4):
                    nc.tensor.matmul(
                        S[0:rows, 512 * c : 512 * (c + 1)],
                        Tp[p][:],
                        mt[:, 512 * c : 512 * (c + 1)],
                        start=(p == 0), stop=(p == PAIRS - 1),
                        skip_group_check=True,
                    )

            # ---- softmax pieces ----
            negmax = small.tile([GRP, 1], F32, tag="negmax")
            nc.vector.tensor_reduce(negmax[:], S[:], axis=AX.X, op=ALU.max,
                                    negate=True)
            W_sb = w_pool.tile([GRP, M], F32, tag="W")
            sumexp = small.tile([GRP, 1], F32, tag="sumexp")
            nc.scalar.activation(W_sb[:], S[:], AF.Exp, bias=negmax[:],
                                 scale=1.0, accum_out=sumexp[:])
            recip = small.tile([GRP, 1], F32, tag="recip")
            nc.vector.reciprocal(recip[:], sumexp[:])

            # ---- gate MLP ----
            # g1 = Wg1[:64].T @ queryT + Wg1[64:65].T @ max_row, accumulated.
            mx_ps = wb_pool.tile([1, GRP], F32, tag="wb")
            nc.tensor.transpose(mx_ps[:], negmax[:], ident[0:GRP, 0:GRP])
            max_row = small.tile([1, GRP], F32, tag="max_row")
            nc.scalar.activation(max_row[:], mx_ps[:], AF.Copy, scale=-1.0)
            g1_ps = wb_pool.tile([GRP, GRP], F32, tag="wb")
            nc.tensor.matmul(g1_ps[:], wg1_sb[:], qt_sb[:, c0 : c0 + GRP],
                             start=True, stop=False, skip_group_check=True)
            nc.tensor.matmul(g1_ps[:], wg1r_sb[:], max_row[:],
                             start=False, stop=True, skip_group_check=True)
            g1_sb = small.tile([GRP, GRP], F32, tag="g1")
            nc.scalar.activation(g1_sb[:], g1_ps[:], AF.Relu, bias=bg1_sb[:])
            g2_ps = wb_pool.tile([1, GRP], F32, tag="wb")
            nc.tensor.matmul(g2_ps[:], wg2_sb[:], g1_sb[:], start=True, stop=True)
            eneg = small.tile([1, GRP], F32, tag="eneg")
            nc.scalar.activation(eneg[:], g2_ps[:], AF.Exp, bias=nbg2_sb[:],
                                 scale=-1.0)
            onep = small.tile([1, GRP], F32, tag="onep")
            nc.vector.tensor_scalar_add(onep[:], eneg[:], 1.0)
            gate_row = small.tile([1, GRP], F32, tag="gate_row")
            nc.vector.reciprocal(gate_row[:], onep[:])
            nc.scalar.dma_start(gate_o[0:1, c0 : c0 + GRP], gate_row[:])

            # per-batch output scale: gate / sumexp, as a [GRP, 1] column
            gc_ps = wb_pool.tile([GRP, 1], F32, tag="wb")
            nc.tensor.matmul(gc_ps[:], gate_row[:], ones1[:], start=True, stop=True)
            gr = small.tile([GRP, 1], F32, tag="gr")
            nc.vector.tensor_tensor(gr[:], gc_ps[:], recip[:], op=ALU.mult)

            # ---- einsum2: retU[d, b] ----
            retc = retc_pool.tile([128, PAIRS], F32, tag="retc")
            for p in range(PAIRS):
                accs = []
                for h in range(2):
                    wb_ps = wb_pool.tile([128, 1024], F32, tag="wb")
                    for c in range(2):
                        nc.tensor.matmul(
                            wb_ps[:, 512 * c : 512 * (c + 1)],
                            sel[:, 128 * p : 128 * (p + 1)],
                            W_sb[:, 1024 * h + 512 * c : 1024 * h + 512 * (c + 1)],
                            start=True, stop=True,
                        )
                    scratch = scr_pool.tile([128, 1024], F32, tag="scr")
                    acc = acc_pool.tile([128, 1], F32, tag="acc")
                    nc.vector.scalar_tensor_tensor(
                        out=scratch[:],
                        in0=mts[p][:, 1024 * h : 1024 * (h + 1)],
                        scalar=1.0,
                        in1=wb_ps[:],
                        op0=ALU.mult,
                        op1=ALU.mult,
                        accum_out=acc[:],
                    )
                    accs.append(acc)
                nc.vector.tensor_tensor(retc[:, p : p + 1], accs[0][:], accs[1][:],
                                        op=ALU.add)

            # gather retU into [64, GRP]: even batches from the top half
            # (same partitions, DVE), odd from the bottom half (partition
            # crossing -> small DMA).
            retT = small.tile([64, GRP], F32, tag="retT")
            nc.vector.tensor_copy(retT[:, 0::2], retc[0:64, :])
            nc.scalar.dma_start(retT[:, 1::2], retc[64:128, :])

            # ---- output matmuls ----
            h1_ps = wb_pool.tile([D, GRP], F32, tag="wb")
            nc.tensor.matmul(h1_ps[:], wo_sb[:], retT[:], start=True, stop=True)
            h1_sb = small.tile([D, GRP], F32, tag="h1")
            nc.scalar.copy(h1_sb[:], h1_ps[:])
            raw_ps = wb_pool.tile([V, GRP], F32, tag="wb")
            nc.tensor.matmul(raw_ps[:], wc_sb[:], h1_sb[:], start=True, stop=True)
            raw_sb = small.tile([V, GRP], F32, tag="raw")
            nc.scalar.copy(raw_sb[:], raw_ps[:])
            rawT_ps = wb_pool.tile([GRP, V], F32, tag="wb")
            nc.tensor.transpose(rawT_ps[:], raw_sb[:], ident[:])
            logits_sb = small.tile([GRP, V], F32, tag="logits")
            nc.vector.tensor_scalar_mul(logits_sb[:], rawT_ps[:], gr[:])
            nc.sync.dma_start(logits_o[c0 : c0 + GRP, :], logits_sb[:])

    nc.compile()
    return nc


def _get_module():
    if "nc" not in _CACHE:
        _CACHE["nc"] = build_module()
    return _CACHE["nc"]


def kernel(query, memory, Wq, bq, Wo, bo, Wg1, bg1, Wg2, bg2, null_vec, Wc, bc):
    global LAST_RESULTS
    query = np.asarray(query, dtype=np.float32)
    memory = np.asarray(memory, dtype=np.float32)
    Wq = np.asarray(Wq, dtype=np.float32)
    bq = np.asarray(bq, dtype=np.float32)
    Wo = np.asarray(Wo, dtype=np.float32)
    bo = np.asarray(bo, dtype=np.float32)
    Wg1 = np.asarray(Wg1, dtype=np.float32)
    bg1 = np.asarray(bg1, dtype=np.float32)
    Wg2 = np.asarray(Wg2, dtype=np.float32)
    bg2 = np.asarray(bg2, dtype=np.float32)
    null_vec = np.asarray(null_vec, dtype=np.float32)
    Wc = np.asarray(Wc, dtype=np.float32)
    bc = np.asarray(bc, dtype=np.float32)

    # host-side layout prep
    memT_full = np.ascontiguousarray(memory.transpose(0, 2, 1))   # [B, D, M]
    qT_full = np.ascontiguousarray(query.T)                       # [H, B]

    shared = {
        "Wq": Wq,
        "bq_s": np.ascontiguousarray((0.125 * bq).reshape(D, 1)),
        "Wo": Wo,
        "Wg1": Wg1,
        "bg1": np.ascontiguousarray(bg1.reshape(32, 1)),
        "Wg2": Wg2,
        "nbg2": np.ascontiguousarray((-bg2).reshape(1, 1)),
        "Wc": Wc,
    }
    in_maps = []
    for c in range(NCORES):
        sl = slice(c * BC, (c + 1) * BC)
        in_maps.append({
            "memT": memT_full[sl].reshape(NPAIR_TOT, 128, M),
            "qT": np.ascontiguousarray(qT_full[:, sl]),
            **shared,
        })

    nc = _get_module()
    res = run_bass_kernel_spmd(nc, in_maps, core_ids=list(range(NCORES)))
    LAST_RESULTS = res

    logits_dev = np.concatenate([r["logits"] for r in res.results], axis=0)
    gate = np.concatenate([r["gate"].reshape(-1) for r in res.results], axis=0)

    # host post-processing: rank-1 bias / null terms
    y = bo @ Wc                                  # [V]
    z = null_vec @ Wc                            # [V]
    logits = logits_dev + gate[:, None] * y[None, :] \
        + (1.0 - gate)[:, None] * z[None, :] + bc[None, :]
    return logits.astype(np.float32), gate.astype(np.float32)
